# revision 12
# baseline (speedup 1.0000x reference)
"""EdgeConv-style GNN message passing kernel for 8 TRN2 NeuronCores.

Computation (per edge e with endpoints row[e], col[e]):
    out0 = edge_attr @ w_self
    out  = out0 * (1 + 0.5*(x[row] @ w_h) + 0.5*(x[col] @ w_t)) + edge_attr
    out  = relu(batchnorm(out))          # BN stats over ALL edges (training mode)

Sharding: edges split evenly across the 8 cores; x and the 128x128
weights replicated.  BN mean/var partials are AllReduce'd across cores
between pass 1 (compute + stats) and pass 2 (normalize + relu).

v2 design (channel-major, bf16, SWDGE transpose-gathers):
  - x table is bf16 [40002, 128] with zero rows at 0 and 40001.  SWDGE
    dma_gather(transpose=True) delivers gathered rows CHANNEL-major
    ([128 ch, n_edges]) straight into SBUF -- no PE transposes at all.
  - int16 gather indices only address 32768 table rows, so the host
    permutes each core's edges into 4 buckets by (row window, col
    window); every gather call uses a single statically-chosen window.
    Bucket capacities are static (padded with dummy edges whose
    edge_attr is zero; they contribute exactly 0 to BN sums).
  - edge_attr is host-transposed to channel-major bf16; out0 and
    (out0 + edge_attr) come from two matmuls against w_self and
    (w_self + I) -- the residual add is free.
  - out_pre stays RESIDENT in SBUF as bf16 (no DRAM scratch).
  - pass 2 is one ACT op per subchunk (scale/bias per partition =
    per channel + Relu), output written channel-major bf16; the host
    transposes back, drops dummies and converts to f32.
"""

import numpy as np
import ml_dtypes

import concourse.bass as bass
import concourse.mybir as mybir
import concourse.tile as tile
from concourse import bacc

P = 128
C = 128
BN_EPS = 1e-5

N_CORES = 8
N_NODES = 40000
N_EDGES = 640000
E_SHARD = N_EDGES // N_CORES  # 80000

# int16 gather windows over the zero-padded table xz[40002]
NZ = N_NODES + 2
LO_ROWS = 32768            # lo window = xz[0:32768]
HI_BASE = NZ - LO_ROWS     # 7234; hi window = xz[7234:40002]
LO_MAX = LO_ROWS - 2       # last node reachable via lo window (32766)

# static bucket capacities (edges per core), multiples of 128:
# (row lo, col lo), (lo, hi), (hi, lo), (hi, hi)
CAPS = (54144, 12160, 12160, 2944)
E_CAP = sum(CAPS)          # 81408

CHUNK = 2048               # edges per gather call
SUB_KB = 4                 # k-blocks (128 edges) per compute subchunk

F32 = mybir.dt.float32
BF16 = mybir.dt.bfloat16
I16 = mybir.dt.int16
AF = mybir.ActivationFunctionType
ALU = mybir.AluOpType

BF16_NP = ml_dtypes.bfloat16


def _chunk_plan():
    """[(e0, ch, row_hi, col_hi)] covering the 4 static buckets."""
    chunks = []
    e0 = 0
    for b, cap in enumerate(CAPS):
        left = cap
        while left:
            ch = min(CHUNK, left)
            assert ch % P == 0
            chunks.append((e0, ch, b >= 2, b % 2 == 1))
            e0 += ch
            left -= ch
    nsub = sum((ch // P + SUB_KB - 1) // SUB_KB for _, ch, _, _ in chunks)
    return chunks, nsub


def build_nc(n_cores=N_CORES):
    chunks, nsub = _chunk_plan()
    nchunk = len(chunks)
    smax = CHUNK // 16

    nc = bacc.Bacc(None, num_devices=n_cores, num_swdge_queues=1)
    xz_t = nc.dram_tensor("xz", [NZ, C], BF16, kind="ExternalInput")
    ea_t = nc.dram_tensor("eaT", [C, E_CAP], BF16, kind="ExternalInput")
    # idxpack[chunk, j, :, :]: j = 0 -> head (row), 1 -> tail (col)
    idx_t = nc.dram_tensor("idxpack", [nchunk, 2, P, smax], I16,
                           kind="ExternalInput")
    wh2_t = nc.dram_tensor("wh2", [C, C], BF16, kind="ExternalInput")
    wt2_t = nc.dram_tensor("wt2", [C, C], BF16, kind="ExternalInput")
    ws_t = nc.dram_tensor("ws", [C, C], BF16, kind="ExternalInput")
    wsi_t = nc.dram_tensor("wsi", [C, C], BF16, kind="ExternalInput")
    gm_t = nc.dram_tensor("gamma", [C, 1], F32, kind="ExternalInput")
    bt_t = nc.dram_tensor("beta", [C, 1], F32, kind="ExternalInput")
    out_t = nc.dram_tensor("outT", [C, E_CAP], BF16, kind="ExternalOutput")

    with tile.TileContext(nc, num_cores=n_cores) as tc:
        with (
            tc.tile_pool(name="constp", bufs=1) as constp,
            tc.tile_pool(name="dramp", bufs=1, space="DRAM") as dramp,
        ):
            wh2 = constp.tile([P, C], BF16)
            nc.sync.dma_start(wh2[:], wh2_t[:, :])
            wt2 = constp.tile([P, C], BF16)
            nc.sync.dma_start(wt2[:], wt2_t[:, :])
            ws = constp.tile([P, C], BF16)
            nc.sync.dma_start(ws[:], ws_t[:, :])
            wsi = constp.tile([P, C], BF16)
            nc.sync.dma_start(wsi[:], wsi_t[:, :])
            gamma_sb = constp.tile([P, 1], F32)
            nc.sync.dma_start(gamma_sb[:], gm_t[:, :])
            beta_sb = constp.tile([P, 1], F32)
            nc.sync.dma_start(beta_sb[:], bt_t[:, :])

            sum_cols = constp.tile([P, nsub], F32)
            sq_cols = constp.tile([P, nsub], F32)
            resident = constp.tile([P, E_CAP], BF16)

            xz_lo = xz_t[0:LO_ROWS, :]
            xz_hi = xz_t[HI_BASE:NZ, :]

            # ---- pass 1 ----
            t_idx = 0
            with (
                tc.tile_pool(name="chunkp", bufs=2) as chunkp,
                tc.tile_pool(name="subp", bufs=2) as subp,
                tc.tile_pool(name="psp", bufs=2, space="PSUM") as psp,
            ):
                for ci, (e0, ch, row_hi, col_hi) in enumerate(chunks):
                    K = ch // P
                    S = ch // 16
                    idx = chunkp.tile([P, 2, smax], I16, tag="idx")
                    nc.sync.dma_start(
                        idx[:, :, 0:S],
                        idx_t[ci, :, :, 0:S].rearrange("j p s -> p j s"),
                    )
                    gxh = chunkp.tile([P, 1, CHUNK], BF16, tag="gxh")
                    gxt = chunkp.tile([P, 1, CHUNK], BF16, tag="gxt")
                    for j, (g, hi) in enumerate(((gxh, row_hi), (gxt, col_hi))):
                        nc.gpsimd.dma_gather(
                            out_ap=g[:, :, 0:ch],
                            in_ap=xz_hi if hi else xz_lo,
                            idxs_ap=idx[:, j, 0:S],
                            num_idxs=ch, num_idxs_reg=ch, elem_size=C,
                            transpose=True, single_packet=False,
                            queue_num=0,
                        )
                    ea_c = chunkp.tile([P, CHUNK], BF16, tag="eac")
                    nc.sync.dma_start(ea_c[:, 0:ch], ea_t[:, e0:e0 + ch])

                    for k0 in range(0, K, SUB_KB):
                        kb = min(SUB_KB, K - k0)
                        F = kb * P
                        f0 = k0 * P
                        s_ps = psp.tile([P, SUB_KB * P], F32, tag="s", bufs=2)
                        nc.tensor.matmul(
                            s_ps[:, 0:F], lhsT=wh2[:], rhs=gxh[:, 0, f0:f0 + F],
                            start=True, stop=False,
                        )
                        nc.tensor.matmul(
                            s_ps[:, 0:F], lhsT=wt2[:], rhs=gxt[:, 0, f0:f0 + F],
                            start=False, stop=True,
                        )
                        o_ps = psp.tile([P, SUB_KB * P], F32, tag="o", bufs=2)
                        nc.tensor.matmul(
                            o_ps[:, 0:F], lhsT=ws[:], rhs=ea_c[:, f0:f0 + F],
                            start=True, stop=True,
                        )
                        op_ps = psp.tile([P, SUB_KB * P], F32, tag="op", bufs=2)
                        nc.tensor.matmul(
                            op_ps[:, 0:F], lhsT=wsi[:], rhs=ea_c[:, f0:f0 + F],
                            start=True, stop=True,
                        )

                        # sx = 0.5*(head + tail); m = out0 * sx
                        sx = subp.tile([P, SUB_KB * P], F32, tag="sx")
                        nc.scalar.activation(sx[:, 0:F], s_ps[:, 0:F], AF.Copy)
                        m = subp.tile([P, SUB_KB * P], F32, tag="m")
                        nc.vector.tensor_tensor(
                            m[:, 0:F], o_ps[:, 0:F], sx[:, 0:F], op=ALU.mult
                        )
                        # out_pre = m + (out0 + ea)
                        res = resident[:, e0 + f0:e0 + f0 + F]
                        nc.vector.tensor_tensor(
                            res, m[:, 0:F], op_ps[:, 0:F], op=ALU.add
                        )
                        nc.vector.tensor_reduce(
                            sum_cols[:, t_idx:t_idx + 1], res,
                            axis=mybir.AxisListType.XY, op=ALU.add,
                        )
                        sqt = subp.tile([P, SUB_KB * P], F32, tag="sqt")
                        nc.scalar.activation(
                            sqt[:, 0:F], res, AF.Square,
                            accum_out=sq_cols[:, t_idx:t_idx + 1],
                        )
                        t_idx += 1
            assert t_idx == nsub

            # ---- BN stats all-reduce + scale/shift ----
            stats2 = constp.tile([P, 2], F32)
            nc.vector.tensor_reduce(
                stats2[:, 0:1], sum_cols[:], axis=mybir.AxisListType.X, op=ALU.add
            )
            nc.vector.tensor_reduce(
                stats2[:, 1:2], sq_cols[:], axis=mybir.AxisListType.X, op=ALU.add
            )
            cc_in = dramp.tile([P, 2], F32)
            nc.sync.dma_start(cc_in[:], stats2[:])
            cc_addr = "Shared" if n_cores > 4 else "Local"
            cc_out = dramp.tile([P, 2], F32, addr_space=cc_addr)
            nc.gpsimd.collective_compute(
                "AllReduce",
                ALU.add,
                replica_groups=[list(range(n_cores))],
                ins=[cc_in[:].opt()],
                outs=[cc_out[:].opt()],
            )
            statsg = constp.tile([P, 2], F32)
            nc.sync.dma_start(statsg[:], cc_out[:])

            inv_e = 1.0 / float(N_EDGES)
            mean = constp.tile([P, 1], F32)
            nc.scalar.mul(mean[:], statsg[:, 0:1], inv_e)
            ex2 = constp.tile([P, 1], F32)
            nc.scalar.mul(ex2[:], statsg[:, 1:2], inv_e)
            msq = constp.tile([P, 1], F32)
            nc.vector.tensor_tensor(msq[:], mean[:], mean[:], op=ALU.mult)
            var = constp.tile([P, 1], F32)
            nc.vector.tensor_tensor(var[:], ex2[:], msq[:], op=ALU.subtract)
            eps_sb = constp.tile([P, 1], F32)
            nc.vector.memset(eps_sb[:], BN_EPS)
            std = constp.tile([P, 1], F32)
            nc.scalar.activation(std[:], var[:], AF.Sqrt, bias=eps_sb[:])
            rstd = constp.tile([P, 1], F32)
            nc.vector.reciprocal(rstd[:], std[:])
            scale = constp.tile([P, 1], F32)
            nc.vector.tensor_tensor(scale[:], gamma_sb[:], rstd[:], op=ALU.mult)
            mscale = constp.tile([P, 1], F32)
            nc.vector.tensor_tensor(mscale[:], mean[:], scale[:], op=ALU.mult)
            shift = constp.tile([P, 1], F32)
            nc.vector.tensor_tensor(shift[:], beta_sb[:], mscale[:], op=ALU.subtract)

            # ---- pass 2: per-channel affine + relu, write channel-major ----
            with tc.tile_pool(name="p2p", bufs=2) as p2p:
                p2i = 0
                for (e0, ch, _, _) in chunks:
                    K = ch // P
                    for k0 in range(0, K, SUB_KB):
                        kb = min(SUB_KB, K - k0)
                        F = kb * P
                        f0 = e0 + k0 * P
                        if p2i % 2 == 0:
                            z = p2p.tile([P, SUB_KB * P], BF16, tag="za")
                            nc.scalar.activation(
                                z[:, 0:F], resident[:, f0:f0 + F], AF.Relu,
                                bias=shift[:], scale=scale[:],
                            )
                        else:
                            z = p2p.tile([P, SUB_KB * P], BF16, tag="zv")
                            nc.vector.tensor_scalar(
                                z[:, 0:F], resident[:, f0:f0 + F],
                                scalar1=scale[:], scalar2=shift[:],
                                op0=ALU.mult, op1=ALU.add,
                            )
                            nc.vector.tensor_scalar_max(z[:, 0:F], z[:, 0:F], 0.0)
                        nc.sync.dma_start(out_t[:, f0:f0 + F], z[:, 0:F])
                        p2i += 1

    if not nc.is_finalized():
        nc.finalize()
    return nc


def _wrap16(a):
    """[n] int array -> dma_gather idx layout [128, n//16] int16."""
    S = a.shape[0] // 16
    w = a.reshape(S, 16).T.astype(np.int16)
    return np.tile(w, (8, 1))


def make_in_maps(x, edge_index, edge_attr, w_self, w_h, w_t, gamma, beta_bn):
    x = np.asarray(x, dtype=np.float32)
    xz = np.zeros((NZ, C), dtype=BF16_NP)
    xz[1:N_NODES + 1] = x.astype(BF16_NP)

    ea = np.asarray(edge_attr, dtype=np.float32)
    ei = np.asarray(edge_index)
    row = ei[0].astype(np.int64)
    col = ei[1].astype(np.int64)

    wh2 = np.ascontiguousarray((0.5 * np.asarray(w_h, np.float32)).astype(BF16_NP))
    wt2 = np.ascontiguousarray((0.5 * np.asarray(w_t, np.float32)).astype(BF16_NP))
    ws_f = np.asarray(w_self, np.float32)
    ws = np.ascontiguousarray(ws_f.astype(BF16_NP))
    wsi = np.ascontiguousarray((ws_f + np.eye(C, dtype=np.float32)).astype(BF16_NP))
    gm = np.ascontiguousarray(np.asarray(gamma, np.float32).reshape(C, 1))
    bt = np.ascontiguousarray(np.asarray(beta_bn, np.float32).reshape(C, 1))

    chunks, _ = _chunk_plan()
    nchunk = len(chunks)
    smax = CHUNK // 16

    in_maps = []
    perms = []
    for cidx in range(N_CORES):
        base = cidx * E_SHARD
        r = row[base:base + E_SHARD]
        c = col[base:base + E_SHARD]
        rhi = r > LO_MAX
        chi = c > LO_MAX
        bucket = rhi.astype(np.int64) * 2 + chi.astype(np.int64)

        # perm[i] = global edge id at padded slot i, or -1 for dummies
        perm = np.full(E_CAP, -1, dtype=np.int64)
        pos = 0
        for b in range(4):
            ids = np.nonzero(bucket == b)[0]
            if len(ids) > CAPS[b]:
                raise RuntimeError(
                    f"bucket {b} overflow on core {cidx}: {len(ids)} > {CAPS[b]}")
            perm[pos:pos + len(ids)] = base + ids
            pos += CAPS[b]
        perms.append(perm)

        valid = perm >= 0
        # channel-major bf16 edge_attr, zeros in dummy slots
        ea_perm = np.zeros((E_CAP, C), dtype=np.float32)
        ea_perm[valid] = ea[perm[valid]]
        eaT = np.ascontiguousarray(ea_perm.T.astype(BF16_NP))

        # per-slot window indices for head/tail
        r_slot = np.zeros(E_CAP, dtype=np.int64)
        c_slot = np.zeros(E_CAP, dtype=np.int64)
        r_slot[valid] = row[perm[valid]]
        c_slot[valid] = col[perm[valid]]

        packs = np.zeros((nchunk, 2, P, smax), dtype=np.int16)
        for ci, (e0, chn, row_hi, col_hi) in enumerate(chunks):
            S = chn // 16
            sl = slice(e0, e0 + chn)
            v = valid[sl]
            for j, (nodes, hi) in enumerate(
                ((r_slot[sl], row_hi), (c_slot[sl], col_hi))
            ):
                if hi:
                    idxv = np.where(v, nodes + 1 - HI_BASE, NZ - 1 - HI_BASE)
                else:
                    idxv = np.where(v, nodes + 1, 0)
                packs[ci, j, :, 0:S] = _wrap16(idxv)
        in_maps.append({
            "xz": xz,
            "eaT": eaT,
            "idxpack": packs,
            "wh2": wh2,
            "wt2": wt2,
            "ws": ws,
            "wsi": wsi,
            "gamma": gm,
            "beta": bt,
        })
    return in_maps, perms


_NC_CACHE = {}


def _get_nc():
    if "nc" not in _NC_CACHE:
        _NC_CACHE["nc"] = build_nc()
    return _NC_CACHE["nc"]


def run(inputs, trace=False, **kwargs):
    from concourse.bass_utils import run_bass_kernel_spmd

    nc = _get_nc()
    in_maps, perms = make_in_maps(
        inputs["x"], inputs["edge_index"], inputs["edge_attr"],
        inputs["w_self"], inputs["w_h"], inputs["w_t"],
        inputs["gamma"], inputs["beta_bn"],
    )
    res = run_bass_kernel_spmd(
        nc, in_maps, core_ids=list(range(N_CORES)), trace=trace, **kwargs
    )
    out = np.empty((N_EDGES, C), dtype=np.float32)
    for i in range(N_CORES):
        outT = np.asarray(res.results[i]["outT"], dtype=np.float32)
        perm = perms[i]
        valid = perm >= 0
        out[perm[valid]] = outT.T[valid]
    return out, res


def kernel(**inputs):
    out, _ = run(inputs, trace=False)
    return out


# revision 14
# speedup vs baseline: 1.1924x; 1.1924x over previous
"""EdgeConv-style GNN message passing kernel for 8 TRN2 NeuronCores.

Computation (per edge e with endpoints row[e], col[e]):
    out0 = edge_attr @ w_self
    out  = out0 * (1 + 0.5*(x[row] @ w_h) + 0.5*(x[col] @ w_t)) + edge_attr
    out  = relu(batchnorm(out))          # BN stats over ALL edges (training mode)

Sharding: edges split evenly across the 8 cores; x and the 128x128
weights replicated.  BN mean/var partials are AllReduce'd across cores
between pass 1 (compute + stats) and pass 2 (normalize + relu).

v2 design (channel-major, bf16, SWDGE transpose-gathers):
  - x table is bf16 [40002, 128] with zero rows at 0 and 40001.  SWDGE
    dma_gather(transpose=True) delivers gathered rows CHANNEL-major
    ([128 ch, n_edges]) straight into SBUF -- no PE transposes at all.
  - int16 gather indices only address 32768 table rows, so the host
    permutes each core's edges into 4 buckets by (row window, col
    window); every gather call uses a single statically-chosen window.
    Bucket capacities are static (padded with dummy edges whose
    edge_attr is zero; they contribute exactly 0 to BN sums).
  - edge_attr is host-transposed to channel-major bf16; out0 and
    (out0 + edge_attr) come from two matmuls against w_self and
    (w_self + I) -- the residual add is free.
  - out_pre stays RESIDENT in SBUF as bf16 (no DRAM scratch).
  - pass 2 is one ACT op per subchunk (scale/bias per partition =
    per channel + Relu), output written channel-major bf16; the host
    transposes back, drops dummies and converts to f32.
"""

import numpy as np
import ml_dtypes

import concourse.bass as bass
import concourse.mybir as mybir
import concourse.tile as tile
from concourse import bacc

P = 128
C = 128
BN_EPS = 1e-5

N_CORES = 8
N_NODES = 40000
N_EDGES = 640000
E_SHARD = N_EDGES // N_CORES  # 80000

# int16 gather windows over the zero-padded table xz[40002]
NZ = N_NODES + 2
LO_ROWS = 32768            # lo window = xz[0:32768]
HI_BASE = NZ - LO_ROWS     # 7234; hi window = xz[7234:40002]
LO_MAX = LO_ROWS - 2       # last node reachable via lo window (32766)

# static bucket capacities (edges per core), multiples of 128:
# (row lo, col lo), (lo, hi), (hi, lo), (hi, hi)
CAPS = (54400, 12416, 12416, 2944)
E_CAP = sum(CAPS)          # 82176

CHUNK = 2048               # edges per gather call
SUB_KB = 4                 # k-blocks (128 edges) per compute subchunk

F32 = mybir.dt.float32
BF16 = mybir.dt.bfloat16
I16 = mybir.dt.int16
AF = mybir.ActivationFunctionType
ALU = mybir.AluOpType

BF16_NP = ml_dtypes.bfloat16


def _chunk_plan():
    """[(e0, ch, row_hi, col_hi)] covering the 4 static buckets."""
    chunks = []
    e0 = 0
    for b, cap in enumerate(CAPS):
        left = cap
        while left:
            ch = min(CHUNK, left)
            assert ch % P == 0
            chunks.append((e0, ch, b >= 2, b % 2 == 1))
            e0 += ch
            left -= ch
    nsub = sum((ch // P + SUB_KB - 1) // SUB_KB for _, ch, _, _ in chunks)
    return chunks, nsub


def build_nc(n_cores=N_CORES):
    chunks, nsub = _chunk_plan()
    nchunk = len(chunks)
    smax = CHUNK // 16

    nc = bacc.Bacc(None, num_devices=n_cores, num_swdge_queues=1)
    xz_t = nc.dram_tensor("xz", [NZ, C], BF16, kind="ExternalInput")
    ea_t = nc.dram_tensor("eaT", [C, E_CAP], BF16, kind="ExternalInput")
    # idxpack[chunk, j, :, :]: j = 0 -> head (row), 1 -> tail (col)
    idx_t = nc.dram_tensor("idxpack", [nchunk, 2, P, smax], I16,
                           kind="ExternalInput")
    wh2_t = nc.dram_tensor("wh2", [C, C], BF16, kind="ExternalInput")
    wt2_t = nc.dram_tensor("wt2", [C, C], BF16, kind="ExternalInput")
    ws_t = nc.dram_tensor("ws", [C, C], BF16, kind="ExternalInput")
    wsi_t = nc.dram_tensor("wsi", [C, C], BF16, kind="ExternalInput")
    gm_t = nc.dram_tensor("gamma", [C, 1], F32, kind="ExternalInput")
    bt_t = nc.dram_tensor("beta", [C, 1], F32, kind="ExternalInput")
    out_t = nc.dram_tensor("outT", [C, E_CAP], BF16, kind="ExternalOutput")

    with tile.TileContext(nc, num_cores=n_cores) as tc:
        with (
            tc.tile_pool(name="constp", bufs=1) as constp,
            tc.tile_pool(name="dramp", bufs=1, space="DRAM") as dramp,
        ):
            wh2 = constp.tile([P, C], BF16)
            nc.sync.dma_start(wh2[:], wh2_t[:, :])
            wt2 = constp.tile([P, C], BF16)
            nc.sync.dma_start(wt2[:], wt2_t[:, :])
            ws = constp.tile([P, C], BF16)
            nc.sync.dma_start(ws[:], ws_t[:, :])
            wsi = constp.tile([P, C], BF16)
            nc.sync.dma_start(wsi[:], wsi_t[:, :])
            gamma_sb = constp.tile([P, 1], F32)
            nc.sync.dma_start(gamma_sb[:], gm_t[:, :])
            beta_sb = constp.tile([P, 1], F32)
            nc.sync.dma_start(beta_sb[:], bt_t[:, :])

            sum_cols = constp.tile([P, nsub], F32)
            sq_cols = constp.tile([P, nsub], F32)
            resident = constp.tile([P, E_CAP], BF16)

            xz_lo = xz_t[0:LO_ROWS, :]
            xz_hi = xz_t[HI_BASE:NZ, :]

            # ---- pass 1 ----
            t_idx = 0
            with (
                tc.tile_pool(name="chunkp", bufs=2) as chunkp,
                tc.tile_pool(name="subp", bufs=2) as subp,
                tc.tile_pool(name="psp", bufs=2, space="PSUM") as psp,
            ):
                for ci, (e0, ch, row_hi, col_hi) in enumerate(chunks):
                    K = ch // P
                    S = ch // 16
                    idx = chunkp.tile([P, 2, smax], I16, tag="idx")
                    nc.sync.dma_start(
                        idx[:, :, 0:S],
                        idx_t[ci, :, :, 0:S].rearrange("j p s -> p j s"),
                    )
                    gxh = chunkp.tile([P, 1, CHUNK], BF16, tag="gxh")
                    gxt = chunkp.tile([P, 1, CHUNK], BF16, tag="gxt")
                    for j, (g, hi) in enumerate(((gxh, row_hi), (gxt, col_hi))):
                        nc.gpsimd.dma_gather(
                            out_ap=g[:, :, 0:ch],
                            in_ap=xz_hi if hi else xz_lo,
                            idxs_ap=idx[:, j, 0:S],
                            num_idxs=ch, num_idxs_reg=ch, elem_size=C,
                            transpose=True, single_packet=False,
                            queue_num=0,
                        )
                    ea_c = chunkp.tile([P, CHUNK], BF16, tag="eac")
                    nc.sync.dma_start(ea_c[:, 0:ch], ea_t[:, e0:e0 + ch])

                    for k0 in range(0, K, SUB_KB):
                        kb = min(SUB_KB, K - k0)
                        F = kb * P
                        f0 = k0 * P
                        s_ps = psp.tile([P, SUB_KB * P], F32, tag="s", bufs=2)
                        nc.tensor.matmul(
                            s_ps[:, 0:F], lhsT=wh2[:], rhs=gxh[:, 0, f0:f0 + F],
                            start=True, stop=False,
                        )
                        nc.tensor.matmul(
                            s_ps[:, 0:F], lhsT=wt2[:], rhs=gxt[:, 0, f0:f0 + F],
                            start=False, stop=True,
                        )
                        o_ps = psp.tile([P, SUB_KB * P], F32, tag="o", bufs=2)
                        nc.tensor.matmul(
                            o_ps[:, 0:F], lhsT=ws[:], rhs=ea_c[:, f0:f0 + F],
                            start=True, stop=True,
                        )
                        op_ps = psp.tile([P, SUB_KB * P], F32, tag="op", bufs=2)
                        nc.tensor.matmul(
                            op_ps[:, 0:F], lhsT=wsi[:], rhs=ea_c[:, f0:f0 + F],
                            start=True, stop=True,
                        )

                        # sx = 0.5*(head + tail); m = out0 * sx
                        sx = subp.tile([P, SUB_KB * P], F32, tag="sx")
                        nc.scalar.activation(sx[:, 0:F], s_ps[:, 0:F], AF.Copy)
                        m = subp.tile([P, SUB_KB * P], F32, tag="m")
                        nc.vector.tensor_tensor(
                            m[:, 0:F], o_ps[:, 0:F], sx[:, 0:F], op=ALU.mult
                        )
                        # out_pre = m + (out0 + ea)
                        res = resident[:, e0 + f0:e0 + f0 + F]
                        nc.vector.tensor_tensor(
                            res, m[:, 0:F], op_ps[:, 0:F], op=ALU.add
                        )
                        nc.vector.tensor_reduce(
                            sum_cols[:, t_idx:t_idx + 1], res,
                            axis=mybir.AxisListType.XY, op=ALU.add,
                        )
                        sqt = subp.tile([P, SUB_KB * P], F32, tag="sqt")
                        nc.scalar.activation(
                            sqt[:, 0:F], res, AF.Square,
                            accum_out=sq_cols[:, t_idx:t_idx + 1],
                        )
                        t_idx += 1
            assert t_idx == nsub

            # ---- BN stats all-reduce + scale/shift ----
            stats2 = constp.tile([P, 2], F32)
            nc.vector.tensor_reduce(
                stats2[:, 0:1], sum_cols[:], axis=mybir.AxisListType.X, op=ALU.add
            )
            nc.vector.tensor_reduce(
                stats2[:, 1:2], sq_cols[:], axis=mybir.AxisListType.X, op=ALU.add
            )
            cc_in = dramp.tile([P, 2], F32)
            nc.sync.dma_start(cc_in[:], stats2[:])
            cc_addr = "Shared" if n_cores > 4 else "Local"
            cc_out = dramp.tile([P, 2], F32, addr_space=cc_addr)
            nc.gpsimd.collective_compute(
                "AllReduce",
                ALU.add,
                replica_groups=[list(range(n_cores))],
                ins=[cc_in[:].opt()],
                outs=[cc_out[:].opt()],
            )
            statsg = constp.tile([P, 2], F32)
            nc.sync.dma_start(statsg[:], cc_out[:])

            inv_e = 1.0 / float(N_EDGES)
            mean = constp.tile([P, 1], F32)
            nc.scalar.mul(mean[:], statsg[:, 0:1], inv_e)
            ex2 = constp.tile([P, 1], F32)
            nc.scalar.mul(ex2[:], statsg[:, 1:2], inv_e)
            msq = constp.tile([P, 1], F32)
            nc.vector.tensor_tensor(msq[:], mean[:], mean[:], op=ALU.mult)
            var = constp.tile([P, 1], F32)
            nc.vector.tensor_tensor(var[:], ex2[:], msq[:], op=ALU.subtract)
            eps_sb = constp.tile([P, 1], F32)
            nc.vector.memset(eps_sb[:], BN_EPS)
            std = constp.tile([P, 1], F32)
            nc.scalar.activation(std[:], var[:], AF.Sqrt, bias=eps_sb[:])
            rstd = constp.tile([P, 1], F32)
            nc.vector.reciprocal(rstd[:], std[:])
            scale = constp.tile([P, 1], F32)
            nc.vector.tensor_tensor(scale[:], gamma_sb[:], rstd[:], op=ALU.mult)
            mscale = constp.tile([P, 1], F32)
            nc.vector.tensor_tensor(mscale[:], mean[:], scale[:], op=ALU.mult)
            shift = constp.tile([P, 1], F32)
            nc.vector.tensor_tensor(shift[:], beta_sb[:], mscale[:], op=ALU.subtract)

            # ---- pass 2: per-channel affine + relu, write channel-major ----
            P2F = 1024
            with tc.tile_pool(name="p2p", bufs=3) as p2p:
                for f0 in range(0, E_CAP, P2F):
                    F = min(P2F, E_CAP - f0)
                    z = p2p.tile([P, P2F], BF16, tag="z")
                    nc.scalar.activation(
                        z[:, 0:F], resident[:, f0:f0 + F], AF.Relu,
                        bias=shift[:], scale=scale[:],
                    )
                    nc.sync.dma_start(out_t[:, f0:f0 + F], z[:, 0:F])

    if not nc.is_finalized():
        nc.finalize()
    return nc


def _wrap16(a):
    """[n] int array -> dma_gather idx layout [128, n//16] int16."""
    S = a.shape[0] // 16
    w = a.reshape(S, 16).T.astype(np.int16)
    return np.tile(w, (8, 1))


def make_in_maps(x, edge_index, edge_attr, w_self, w_h, w_t, gamma, beta_bn):
    x = np.asarray(x, dtype=np.float32)
    xz = np.zeros((NZ, C), dtype=BF16_NP)
    xz[1:N_NODES + 1] = x.astype(BF16_NP)

    ea = np.asarray(edge_attr, dtype=np.float32)
    ei = np.asarray(edge_index)
    row = ei[0].astype(np.int64)
    col = ei[1].astype(np.int64)

    wh2 = np.ascontiguousarray((0.5 * np.asarray(w_h, np.float32)).astype(BF16_NP))
    wt2 = np.ascontiguousarray((0.5 * np.asarray(w_t, np.float32)).astype(BF16_NP))
    ws_f = np.asarray(w_self, np.float32)
    ws = np.ascontiguousarray(ws_f.astype(BF16_NP))
    wsi = np.ascontiguousarray((ws_f + np.eye(C, dtype=np.float32)).astype(BF16_NP))
    gm = np.ascontiguousarray(np.asarray(gamma, np.float32).reshape(C, 1))
    bt = np.ascontiguousarray(np.asarray(beta_bn, np.float32).reshape(C, 1))

    chunks, _ = _chunk_plan()
    nchunk = len(chunks)
    smax = CHUNK // 16

    in_maps = []
    perms = []
    for cidx in range(N_CORES):
        base = cidx * E_SHARD
        r = row[base:base + E_SHARD]
        c = col[base:base + E_SHARD]
        rhi = r > LO_MAX
        chi = c > LO_MAX
        bucket = rhi.astype(np.int64) * 2 + chi.astype(np.int64)

        # perm[i] = global edge id at padded slot i, or -1 for dummies
        perm = np.full(E_CAP, -1, dtype=np.int64)
        pos = 0
        for b in range(4):
            ids = np.nonzero(bucket == b)[0]
            if len(ids) > CAPS[b]:
                raise RuntimeError(
                    f"bucket {b} overflow on core {cidx}: {len(ids)} > {CAPS[b]}")
            perm[pos:pos + len(ids)] = base + ids
            pos += CAPS[b]
        perms.append(perm)

        valid = perm >= 0
        # channel-major bf16 edge_attr, zeros in dummy slots
        ea_perm = np.zeros((E_CAP, C), dtype=np.float32)
        ea_perm[valid] = ea[perm[valid]]
        eaT = np.ascontiguousarray(ea_perm.T.astype(BF16_NP))

        # per-slot window indices for head/tail
        r_slot = np.zeros(E_CAP, dtype=np.int64)
        c_slot = np.zeros(E_CAP, dtype=np.int64)
        r_slot[valid] = row[perm[valid]]
        c_slot[valid] = col[perm[valid]]

        packs = np.zeros((nchunk, 2, P, smax), dtype=np.int16)
        for ci, (e0, chn, row_hi, col_hi) in enumerate(chunks):
            S = chn // 16
            sl = slice(e0, e0 + chn)
            v = valid[sl]
            for j, (nodes, hi) in enumerate(
                ((r_slot[sl], row_hi), (c_slot[sl], col_hi))
            ):
                if hi:
                    idxv = np.where(v, nodes + 1 - HI_BASE, NZ - 1 - HI_BASE)
                else:
                    idxv = np.where(v, nodes + 1, 0)
                packs[ci, j, :, 0:S] = _wrap16(idxv)
        in_maps.append({
            "xz": xz,
            "eaT": eaT,
            "idxpack": packs,
            "wh2": wh2,
            "wt2": wt2,
            "ws": ws,
            "wsi": wsi,
            "gamma": gm,
            "beta": bt,
        })
    return in_maps, perms


_NC_CACHE = {}


def _get_nc():
    if "nc" not in _NC_CACHE:
        _NC_CACHE["nc"] = build_nc()
    return _NC_CACHE["nc"]


def run(inputs, trace=False, **kwargs):
    from concourse.bass_utils import run_bass_kernel_spmd

    nc = _get_nc()
    in_maps, perms = make_in_maps(
        inputs["x"], inputs["edge_index"], inputs["edge_attr"],
        inputs["w_self"], inputs["w_h"], inputs["w_t"],
        inputs["gamma"], inputs["beta_bn"],
    )
    res = run_bass_kernel_spmd(
        nc, in_maps, core_ids=list(range(N_CORES)), trace=trace, **kwargs
    )
    out = np.empty((N_EDGES, C), dtype=np.float32)
    for i in range(N_CORES):
        outT = np.asarray(res.results[i]["outT"], dtype=np.float32)
        perm = perms[i]
        valid = perm >= 0
        out[perm[valid]] = outT.T[valid]
    return out, res


def kernel(**inputs):
    out, _ = run(inputs, trace=False)
    return out
